# revision 6
# baseline (speedup 1.0000x reference)
"""Max-pooling over sequence spans — Trainium2 Bass kernel, v2.

Problem: context [B=8, S=4096, H=1024] f32; spans_begin/spans_len [B, 100] i32.
Output [B, 100, H] f32: out[b, n] = max over rows context[b, begin:begin+max(len,1)].
Pure data-parallel over batch: one batch row per NeuronCore, 8 cores.

v2 design (vs the 64-step row-gather baseline):
  * bf16 context: the host uploads context as bf16 (half the gather read
    bytes); accumulation runs in bf16 on the DVE 2x mode.  rel err ~2^-9,
    well under the 2e-2 gate.
  * Exact-traffic gather: spans are sorted desc and packed into a
    128-partition x 18-slot grid (2 rows per slot chunk, 2 accumulator
    chains A/B = even/odd slots).  A span owns 1-4 cells: a cell is one
    chain of one partition (up to 9 chunks = 18 rows).  Cells of one span
    pair up A∧B on the same partition where possible; those partitions sit
    in a depth-sorted prefix merged by ONE tensor_tensor over [0, n1m).
    Leftover single cells pair two-different-spans per partition in the
    suffix; the host max-combines each span's 1-3 partial rows while
    unsharding (same host pass that undoes the sort permutation).
  * Per-slot partition cutoffs [0, P_g) with depth-sorted layout keep
    gathered bytes close to sum(ceil(eff/2)*2).
  * Chunked descriptors: each slot gathers 2 contiguous rows per partition
    in one descriptor; slot instructions alternate between 2 SWDGE queues.
  * eff==1 spans bypass compute: their row is gathered and written straight
    to the output via a casting DMA.

Compile is per-input-schedule (cached in-process, like the baseline's
value-dependent split schedule).
"""

import sys
import numpy as np

sys.path.insert(0, "/opt/trn_rl_repo")

B, S, H = 8, 4096, 1024
N_SPANS = 100
MAX_LEN = 64
N_CORES = 8

C = 2            # rows per chunk (one descriptor)
CAP_A = 9        # slots in chain A
CAP_B = 8        # slots in chain B (asymmetric: saves a full slot row)
SLOTS = CAP_A + CAP_B       # 17 gather slots
CAPR = (CAP_A * C, CAP_B * C)   # rows per cell by chain: (18, 16)


def slot_col(c, gc):
    """Map (chain, chain-slot) to the interleaved global slot column."""
    return 2 * gc + c if gc < CAP_B or c == 1 else 2 * CAP_B + (gc - CAP_B)

_cache = {}


# --------------------------------------------------------------------------
# schedule builder (host)
# --------------------------------------------------------------------------

def build_schedule(spans_begin, spans_len):
    """Pack spans into the cell grid.  Returns schedule dict or None."""
    spans_begin = np.asarray(spans_begin, np.int64)
    eff = np.maximum(np.asarray(spans_len, np.int64), 1)
    assert eff.shape == (B, N_SPANS)
    if eff.max() > 2 * (CAPR[0] + CAPR[1]):
        return None

    is1 = eff == 1
    n1v = int(is1.sum(axis=1).max())
    M = int((~is1).sum(axis=1).max())

    eff_s = np.full((B, M), 2, np.int64)
    beg_s = np.zeros((B, M), np.int64)
    orders = []
    d1_src = []
    d1_off = np.zeros((B, max(n1v, 1)), np.int64)
    for b in range(B):
        nt = np.where(~is1[b])[0]
        o = nt[np.argsort(-eff[b][nt], kind="stable")]
        orders.append(o)
        eff_s[b, :len(o)] = eff[b][o]
        beg_s[b, :len(o)] = spans_begin[b][o]
        t1 = np.where(is1[b])[0]
        d1_src.append(t1)
        d1_off[b, :len(t1)] = spans_begin[b][t1]

    prof = eff_s.max(axis=0)          # virtual profile, non-increasing

    # cells per rank: smallest k whose cap sum covers prof.  Cell caps come
    # in (A, B) pairs: [18, 16, 18, 16] -> cumsum [18, 34, 52, 68]
    cell_caps = [CAPR[0], CAPR[1], CAPR[0], CAPR[1]]
    cums = np.cumsum(cell_caps)
    k_of = np.searchsorted(cums, prof) + 1
    if prof.max() > cums[-1]:
        return None

    def split_parts(v, k):
        """Split v rows into k parts, part i <= cell_caps[i], each >= min
        (balanced where caps allow)."""
        v = int(v)
        caps = cell_caps[:k]
        parts = []
        left = v
        for i in range(k):
            rest = k - i - 1
            p = min(caps[i], max(-(-left // (rest + 1)),
                                 left - sum(caps[i + 1:k])))
            parts.append(p)
            left -= p
        return parts

    # partition units: pairs (A+B cells of one rank) in the m1 prefix;
    # leftover single cells pack into suffix partitions (greedy by depth:
    # deep singles >CAP_B chunks must take an A cell)
    pair_units = []    # (depth, rank, part_a, part_b)
    single_cells = []  # (depth, rank, part_idx)
    for r in range(M):
        k = int(k_of[r])
        plens = split_parts(prof[r], k)
        depths = [-(-pl // C) for pl in plens]
        for i in range(0, k - (k % 2), 2):
            pair_units.append((max(depths[i], depths[i + 1]), r, i, i + 1))
        if k % 2:
            single_cells.append((depths[k - 1], r, k - 1))

    pair_units.sort(key=lambda t: -t[0])
    single_cells.sort(key=lambda t: -t[0])

    n1m = len(pair_units)
    rank_cells = [[None] * int(k_of[r]) for r in range(M)]
    for p, (_, r, ia, ib) in enumerate(pair_units):
        rank_cells[r][ia] = (p, 0)
        rank_cells[r][ib] = (p, 1)
    # two-pointer single packing: deepest takes the A cell, pairs with the
    # shallowest remaining on the B cell (B holds at most CAP_B chunks)
    lo, hi = 0, len(single_cells) - 1
    p_cur = n1m
    while lo <= hi:
        _, r, i = single_cells[lo]
        rank_cells[r][i] = (p_cur, 0)
        if hi > lo:
            depth_b, rb, ib = single_cells[hi]
            if depth_b > CAP_B:
                return None
            rank_cells[rb][ib] = (p_cur, 1)
            hi -= 1
        lo += 1
        p_cur += 1
    P_total = p_cur
    if P_total > 128:
        return None

    # virtual chunk counts per (chain, partition)
    nchunk = np.zeros((2, 128), np.int64)
    for r in range(M):
        plens = split_parts(prof[r], int(k_of[r]))
        for (p, c), pl in zip(rank_cells[r], plens):
            nchunk[c, p] = -(-pl // C)
    assert nchunk[0].max() <= CAP_A and nchunk[1].max() <= CAP_B

    Pg = np.zeros((2, CAP_A), np.int64)
    for c in range(2):
        for g in range(CAP_A if c == 0 else CAP_B):
            occ = np.where(nchunk[c] > g)[0]
            Pg[c, g] = (int(occ.max()) + 1) if len(occ) else 0

    # per-batch chunk offsets.  All slots gather the FULL 128 partitions
    # (partial-partition gathers measured ~4x slower), so every (partition,
    # slot) needs a valid offset.  Padding reads are spread over distinct
    # rows (duplicate addresses in one gather measured ~2-3x slower):
    #   - unused (partition, slot) entries walk dummy rows spread over S;
    #   - a cell's unused trailing slots walk windows spread over the WHOLE
    #     span (any in-span rows are harmless: cells are max-combined).
    spread = (np.arange(128, dtype=np.int64)[:, None] * 131
              + np.arange(SLOTS + 1, dtype=np.int64)[None, :] * 17) % (S - C)
    idxs = np.broadcast_to(spread[None], (B, 128, SLOTS + 1)).astype(np.int32).copy()
    for bi in range(B):
        for r in range(M):
            k = int(k_of[r])
            v = int(eff_s[bi, r])
            a0 = int(beg_s[bi, r])
            wspan = max(v - C, 0)           # span-wide pad window starts
            kk = max(min(k, v // C), 1)
            parts = []
            pos = 0
            for pl in split_parts(v, kk):   # caps-aware: part i <= caps[i]
                parts.append((a0 + pos, pl))
                pos += pl
            while len(parts) < k:           # unused cells re-read part 0
                parts.append(parts[0])
            for ci, ((p, c), (pa, pl)) in enumerate(zip(rank_cells[r], parts)):
                span = max(pl - C, 0)
                nch = max(-(-pl // C), 1)
                ncap = CAP_A if c == 0 else CAP_B
                for g in range(ncap):
                    if g < nch:
                        o = pa + min(g * C, span)
                    else:
                        o = a0 + (p * 3 + g * 5) % (wspan + 1)
                    idxs[bi, p, slot_col(c, g)] = o
        idxs[bi, :d1_off.shape[1], SLOTS] = d1_off[bi]

    gathered = 128 * SLOTS * C
    return dict(
        n1v=n1v, M=M, prof=prof, Pg=Pg, n1m=n1m, P_total=P_total,
        idxs=idxs, orders=orders, d1_src=d1_src, rank_cells=rank_cells,
        gathered_rows=gathered,
        struct_key=(n1v, M, n1m, tuple(Pg.ravel().tolist())),
    )


# --------------------------------------------------------------------------
# program builder
# --------------------------------------------------------------------------

def build_program_v2(sched, repeat=1, n_slab_bufs=8, n_queues=4):
    import concourse.bass as bass
    import concourse.bacc as bacc
    import concourse.mybir as mybir
    import concourse.tile as tile

    n1v = sched["n1v"]
    Pg = sched["Pg"]
    n1m = sched["n1m"]
    n1c = max(n1v, 1)

    nc = bacc.Bacc("TRN2", target_bir_lowering=False, debug=False,
                   num_devices=N_CORES,
                   num_swdge_queues=max(n_queues, 1))
    ctx_d = nc.dram_tensor("ctx", [S, H], mybir.dt.bfloat16,
                           kind="ExternalInput")
    idx_d = nc.dram_tensor("idx", [128, SLOTS + 1], mybir.dt.int32,
                           kind="ExternalInput")
    # outputs stay bf16 on device; the host casts to f32 while unsharding
    out_d = nc.dram_tensor("out", [256 + n1c, H], mybir.dt.bfloat16,
                           kind="ExternalOutput")

    with tile.TileContext(nc) as tc:
        with (
            tc.tile_pool(name="persist", bufs=1) as persist,
            tc.tile_pool(name="slabs", bufs=n_slab_bufs) as slabs,
        ):
            idx_t = persist.tile([128, SLOTS + 1], mybir.dt.int32)
            nc.sync.dma_start(out=idx_t[:], in_=idx_d[:])
            # double-buffered accumulators: iteration r+1's init does not
            # wait for iteration r's output DMA to drain
            accA0 = persist.tile([128, H], mybir.dt.bfloat16, tag="accA0")
            accB0 = persist.tile([128, H], mybir.dt.bfloat16, tag="accB0")
            accA1 = persist.tile([128, H], mybir.dt.bfloat16, tag="accA1")
            accB1 = persist.tile([128, H], mybir.dt.bfloat16, tag="accB1")
            acc_pairs = [(accA0, accB0), (accA1, accB1)]
            for rep in range(repeat):
                accA, accB = acc_pairs[rep % 2]
                accs = [accA, accB]
                # direct tier (eff==1): bf16 row straight to out
                if n1v > 0:
                    dslab = slabs.tile([n1v, H], mybir.dt.bfloat16)
                    nc.gpsimd.indirect_dma_start(
                        out=dslab[:], out_offset=None, in_=ctx_d[:],
                        in_offset=bass.IndirectOffsetOnAxis(
                            ap=idx_t[0:n1v, SLOTS:SLOTS + 1], axis=0))
                    nc.sync.dma_start(out=out_d[256:256 + n1v, :],
                                      in_=dslab[:])
                # slot gathers + folds (always the full 128 partitions).
                # No device-side chain merge: the host max-combines a span's
                # cells while unsharding, so each chain's output DMA fires as
                # soon as its own last fold completes.
                for g in range(SLOTS):
                    c = g % 2
                    gc = g // 2
                    slab = slabs.tile([128, C * H], mybir.dt.bfloat16)
                    inst = nc.gpsimd.indirect_dma_start(
                        out=slab[:], out_offset=None, in_=ctx_d[:],
                        in_offset=bass.IndirectOffsetOnAxis(
                            ap=idx_t[:, g:g + 1], axis=0))
                    q = g % n_queues
                    if q:
                        inst.ins.queue = f"qPoolDynamic{q}"
                    acc = accs[c]
                    if gc == 0:
                        nc.vector.tensor_copy(out=acc[:], in_=slab[:, 0:H])
                    else:
                        nc.vector.tensor_tensor(
                            out=acc[:], in0=acc[:],
                            in1=slab[:, 0:H], op=mybir.AluOpType.max)
                    nc.vector.tensor_tensor(
                        out=acc[:], in0=acc[:],
                        in1=slab[:, H:C * H], op=mybir.AluOpType.max)
                    if g == SLOTS - 2:      # chain B's last slot
                        nc.sync.dma_start(out=out_d[128:256, :], in_=accB[:])
                nc.sync.dma_start(out=out_d[0:128, :], in_=accA[:])
    nc.compile()
    return nc


# --------------------------------------------------------------------------
# baseline fallback (unsplit row-gather, bit-exact)
# --------------------------------------------------------------------------

def _build_fallback(n_steps, repeat=1):
    import concourse.bass as bass
    import concourse.bacc as bacc
    import concourse.mybir as mybir
    import concourse.tile as tile

    nc = bacc.Bacc("TRN2", target_bir_lowering=False, debug=False,
                   num_devices=N_CORES)
    ctx_d = nc.dram_tensor("ctx", [S, H], mybir.dt.float32, kind="ExternalInput")
    idx_d = nc.dram_tensor("idx", [N_SPANS, n_steps], mybir.dt.int32,
                           kind="ExternalInput")
    out_d = nc.dram_tensor("out", [N_SPANS, H], mybir.dt.float32,
                           kind="ExternalOutput")
    with tile.TileContext(nc) as tc:
        with (
            tc.tile_pool(name="persist", bufs=1) as persist,
            tc.tile_pool(name="slabs", bufs=6) as slabs,
        ):
            idx_t = persist.tile([N_SPANS, n_steps], mybir.dt.int32)
            nc.sync.dma_start(out=idx_t[:], in_=idx_d[:])
            for _ in range(repeat):
                accs = []
                for k in range(2):
                    acc = persist.tile([N_SPANS, H], mybir.dt.float32,
                                       tag=f"acc{k}")
                    nc.vector.memset(acc[:], -3.0e38)
                    accs.append(acc)
                for l in range(n_steps):
                    slab = slabs.tile([N_SPANS, H], mybir.dt.float32)
                    nc.gpsimd.indirect_dma_start(
                        out=slab[:], out_offset=None, in_=ctx_d[:],
                        in_offset=bass.IndirectOffsetOnAxis(
                            ap=idx_t[:, l:l + 1], axis=0))
                    acc = accs[l % 2]
                    nc.vector.tensor_tensor(out=acc[:], in0=acc[:],
                                            in1=slab[:],
                                            op=mybir.AluOpType.max)
                nc.vector.tensor_tensor(out=accs[0][:], in0=accs[0][:],
                                        in1=accs[1][:],
                                        op=mybir.AluOpType.max)
                nc.sync.dma_start(out=out_d[:], in_=accs[0][:])
    nc.compile()
    return nc


def _make_indices(spans_begin, spans_len, n_steps=MAX_LEN):
    eff = np.maximum(spans_len, 1)
    steps = np.arange(n_steps, dtype=np.int32)
    idx = spans_begin[:, :, None] + np.minimum(steps[None, None, :],
                                               eff[:, :, None] - 1)
    return np.clip(idx, 0, S - 1).astype(np.int32)


# --------------------------------------------------------------------------
# host decode (unshard: undo sort, max-combine a span's partial cells)
# --------------------------------------------------------------------------

def decode_output(sched, res_list):
    out = np.empty((B, N_SPANS, H), np.float32)
    n1v = sched["n1v"]
    n1m = sched["n1m"]
    rank_cells = sched["rank_cells"]
    for b in range(B):
        o = np.asarray(res_list[b]["out"]).astype(np.float32)
        order = sched["orders"][b]
        for r in range(len(order)):
            # max-combine all of the span's cells (no device-side merge)
            acc = None
            for p, c in rank_cells[r]:
                row = o[c * 128 + p]
                acc = row if acc is None else np.maximum(acc, row)
            out[b, order[r]] = acc
        for i, sp in enumerate(sched["d1_src"][b]):
            out[b, sp] = o[256 + i]
    return out


# --------------------------------------------------------------------------
# entry point
# --------------------------------------------------------------------------

def _spot_check(out, context, spans_begin, spans_len, tol=0.02):
    """Cheap host-side validation of a sample of spans (guards against the
    occasional garbage first execution of a freshly loaded program)."""
    eff = np.maximum(spans_len, 1)
    rng = np.random.default_rng(12345)
    for b in range(B):
        for n in rng.integers(0, N_SPANS, size=12):
            a = int(spans_begin[b, n])
            exp = context[b, a:a + int(eff[b, n])].max(axis=0)
            rel = np.abs(out[b, n] - exp) / np.maximum(np.abs(exp), 1e-6)
            if rel.max() > tol:
                return False
    return True


def kernel(context, spans_begin, spans_len):
    from concourse.bass_utils import run_bass_kernel_spmd

    context = np.ascontiguousarray(context, dtype=np.float32)
    spans_begin = np.asarray(spans_begin, dtype=np.int32)
    spans_len = np.asarray(spans_len, dtype=np.int32)
    assert context.shape == (B, S, H), context.shape
    assert spans_begin.shape == (B, N_SPANS), spans_begin.shape

    sched = build_schedule(spans_begin, spans_len)
    if sched is not None:
        import ml_dtypes
        ctx16 = context.astype(ml_dtypes.bfloat16)
        key = ("v2", sched["struct_key"])
        if key not in _cache:
            _cache[key] = build_program_v2(sched)
        nc = _cache[key]
        in_maps = [{"ctx": ctx16[b], "idx": sched["idxs"][b]}
                   for b in range(B)]
        for _ in range(3):
            res = run_bass_kernel_spmd(nc, in_maps, list(range(N_CORES)))
            out = decode_output(sched, res.results)
            if _spot_check(out, context, spans_begin, spans_len):
                return out
        # fall through to the bit-exact fallback if the packed program
        # keeps producing bad results

    # fallback: bit-exact row-gather
    n_steps = int(min(MAX_LEN, max(1, np.maximum(spans_len, 1).max())))
    idx = _make_indices(spans_begin, spans_len, n_steps)
    key = ("fb", n_steps)
    if key not in _cache:
        _cache[key] = _build_fallback(n_steps)
    nc = _cache[key]
    in_maps = [{"ctx": context[b], "idx": idx[b]} for b in range(B)]
    res = run_bass_kernel_spmd(nc, in_maps, list(range(N_CORES)))
    out = np.stack([res.results[b]["out"] for b in range(B)], axis=0)
    return out.astype(np.float32)
